# revision 8
# baseline (speedup 1.0000x reference)
"""Trainium2 Bass kernel for a 1-layer transformer encoder block (v2).

Reference (B=4, T=1024, E=1024, H=16, DH=64):
    x1 = LN(x);  q/k/v per-head projections of x1
    attn = softmax(q @ k^T * T**-0.5);  ctx = attn @ v (concat heads)
    x2 = LN(x + ctx);  x2 = x2 + x2 @ ffw + ffb;  out = LN(x2)
    also returns attn[:, -1] (head 15's full map)

Sharding: 8 cores = (batch b, token-half).  Each core owns 512 query
tokens of one batch; k/v are computed for the full batch (duplicated
across the pair of cores sharing a batch) so no collectives are needed.
Token order on device is rotated so own tokens are always cols 0:512.

v2 design notes (vs v1):
- ZERO GpSimd ops.  HW gpsimd ops cost multi-us each (two software
  dispatch hops + TIE-FIFO streaming); v1 spent ~200 Pool-engine ops and
  measured 2.4x its simulated time.  All elementwise now on DVE/ACT,
  all partition-broadcasts are K=1 PE matmuls (ones-row x stat-row ->
  PSUM -> one DVE copy to SBUF).
- LN1 and LN2 are FOLDED INTO the QKV / FFN matmuls:
    LN(x) @ W  ==  rsig_t * [ (x @ W')  +  nmu_t (x) colsum(W')
                              + sig_t (x) bias_row ]
  with W' = g (.) W, bias_row = b@W + b_proj, nmu = -mean, sig = 1/rsig,
  all per-token rows computed on device and applied as K=1 rank-1 matmul
  accumulations into the same PSUM group.  The only elementwise cost per
  projection chunk is one DVE op (the *rsig_t scale), and the x1/x2n
  tensors are never materialized.
- x ships in bf16 (matmul operand + residual + stats all read bf16).
- Softmax denominator via ones-columns packed into the V operand
  (unchanged from v1): per head pair the V buffer holds
  [v_even(64) | one_e | one_o | gap(31) | v_odd(64)]; even head ctx
  window cols 0:128 (Z at row 64), odd head cols 33:161 (Z at row 32).
- rsqrt = exp(-0.5*ln(var+eps)); sig = exp(+0.5*ln(var+eps)).
"""

import numpy as np
import ml_dtypes

import concourse.bass as bass
from concourse import bacc
import concourse.mybir as mybir
import concourse.tile as tile

B, T, E, H, DH = 4, 1024, 1024, 16, 64
P = 128
EC = E // P          # 8 feature chunks
SC = T // P          # 8 key-token chunks
TOWN = T // 2        # 512 own query tokens per core
EPS = 1e-5
SCORE_SCALE = T ** -0.5   # 1/32 (fp8: /WSCALE^2 folded in at exp)
VW = 164             # packed v-pair window width (161 used)

F32 = mybir.dt.float32
BF16 = mybir.dt.bfloat16
AF = mybir.ActivationFunctionType
ALU = mybir.AluOpType

NBF = ml_dtypes.bfloat16

FP8 = True                # fp8e4 + DoubleRow for QKV/V projection matmuls
FP8_FFN = False           # FFN stays bf16: fp8 x2 puts ~6% on the direct
                          # residual path into `out` (matmul rel-err == input
                          # vector rel-err; it does not average down)
F8 = mybir.dt.float8e4
NF8 = ml_dtypes.float8_e4m3
WSCALE = 64.0             # fp8 weight scale (weights are ~N(0, 0.02^2))


def _patched_act_tables(module_arch):
    """Restrict Exp/Ln to the one table set containing both, so the
    act-table-load pass emits a single set id instead of thrashing."""
    import concourse.hw_specs as hw_specs
    tabs = hw_specs.get_activation_tables(module_arch)
    both = [k for k, v in tabs.items()
            if AF.Exp in v and AF.Ln in v]
    if not both:
        return tabs
    keep = both[0]
    out = {}
    for k, v in tabs.items():
        out[k] = v if k == keep else (v - {AF.Exp, AF.Ln})
    return out


def build_nc(reps=1, has_bias=True):
    """Build the kernel module.  reps>1 wraps the per-invocation body
    (input DMA + compute + output DMA) in a hardware For_i loop running it
    `reps` times; iterations are identical so outputs are unchanged.  Used
    by run_timed to measure steady-state per-invocation HW time."""
    nc = bacc.Bacc(None, target_bir_lowering=False)
    _orig_tables = bacc.get_activation_tables
    bacc.get_activation_tables = _patched_act_tables

    # ---- dram I/O ----
    x_own_d = nc.dram_tensor("xT_own", [E, TOWN], BF16, kind="ExternalInput")
    x_oth_d = nc.dram_tensor("xT_oth", [E, TOWN], BF16, kind="ExternalInput")
    WDT = F8 if FP8 else BF16
    wq_d = nc.dram_tensor("wq_b", [EC, P, EC, P], WDT, kind="ExternalInput")
    wk_d = nc.dram_tensor("wk_b", [EC, P, EC, P], WDT, kind="ExternalInput")
    wv_d = nc.dram_tensor("wv_b", [2, P, EC, 512], WDT, kind="ExternalInput")  # parity-major
    FDT = F8 if FP8_FFN else BF16
    ffw_d = nc.dram_tensor("ffw_b", [EC, P, EC, P], FDT, kind="ExternalInput")  # g2-folded
    if FP8:
        x8o_d = nc.dram_tensor("x8_own", [E, TOWN], F8, kind="ExternalInput")
        x8h_d = nc.dram_tensor("x8_oth", [E, TOWN], F8, kind="ExternalInput")
        x8o_view = x8o_d.ap().rearrange("(c p) t -> p c t", p=P)
        x8h_view = x8h_d.ap().rearrange("(c p) t -> p c t", p=P)
    # bf16 row constants:
    # [csq, bq_eff, csk, bk_eff, csv, bv_eff, csf, bf_eff, g3, b3]
    rows_d = nc.dram_tensor("rows_b", [1, 10 * E], BF16, kind="ExternalInput")
    # packed per-partition f32 constants: [g2, g3, b3]
    cst_d = nc.dram_tensor("cst_p", [P, 3 * EC], F32, kind="ExternalInput")

    outT_d = nc.dram_tensor("outT", [E, TOWN], BF16, kind="ExternalOutput")
    a15_d = nc.dram_tensor("attn15T", [T, TOWN], BF16, kind="ExternalOutput")

    xo_view = x_own_d.ap().rearrange("(c p) t -> p c t", p=P)
    xh_view = x_oth_d.ap().rearrange("(c p) t -> p c t", p=P)
    out_view = outT_d.ap().rearrange("(c p) t -> p c t", p=P)
    a15_view = a15_d.ap().rearrange("(c p) t -> p c t", p=P)

    with tile.TileContext(nc) as tc:
        with (
            tc.tile_pool(name="const", bufs=1) as const,
            tc.tile_pool(name="big", bufs=1) as big,
            tc.tile_pool(name="wpool", bufs=4) as wpool,
            tc.tile_pool(name="tmp", bufs=2) as tmp,
            tc.tile_pool(name="rowp", bufs=2) as rowp,
            tc.tile_pool(name="psum", bufs=1, space="PSUM") as psum,
        ):
            # ---- constants (outside the timing loop) ----
            ones_cb = const.tile([P, 1], BF16)       # stat matmul lhsT (bf16)
            nc.vector.memset(ones_cb, 1.0)
            onesP = const.tile([P, P], F32)          # bcast lhsT rows (any partition)
            nc.vector.memset(onesP, 1.0)
            ones1 = const.tile([P, 1], F32)          # rhs for row->col transpose
            nc.vector.memset(ones1, 1.0)
            eps1 = const.tile([1, 1], F32)
            nc.vector.memset(eps1, EPS)
            cst = const.tile([P, 3 * EC], F32)
            nc.sync.dma_start(cst, cst_d.ap())
            g2_p = cst[:, 0 * EC:1 * EC]
            g3_p = cst[:, 1 * EC:2 * EC]
            b3_p = cst[:, 2 * EC:3 * EC]
            rows = const.tile([1, 10 * E], BF16)
            nc.sync.dma_start(rows, rows_d.ap())
            csq_r = rows[:, 0 * E:1 * E]
            bq_r = rows[:, 1 * E:2 * E]
            csk_r = rows[:, 2 * E:3 * E]
            bk_r = rows[:, 3 * E:4 * E]
            csv_r = rows[:, 4 * E:5 * E]
            bv_r = rows[:, 5 * E:6 * E]
            csf_r = rows[:, 6 * E:7 * E]
            bf_r = rows[:, 7 * E:8 * E]
            g3_r = rows[:, 8 * E:9 * E]
            b3_r = rows[:, 9 * E:10 * E]
            ones_row = const.tile([1, 512], BF16)
            nc.vector.memset(ones_row, 1.0)
            invsP = const.tile([1, P], F32)   # bcast lhsT carrying 1/WSCALE
            nc.vector.memset(invsP, (1.0 / WSCALE) if FP8_FFN else 1.0)

            def pmm(name, width=512):
                # main accumulation ring (QKV / V / FFN chains + bcasts +
                # attention scores).  Slots are [P,1024] (2 banks); most
                # users take a [P,512] slice.
                t = psum.tile([P, 1024], F32, tag="mm", bufs=2, name=name)
                return t[:, 0:width]

            def pst(name):
                # stat-chain ring [1,512]
                return psum.tile([1, 512], F32, tag="st", bufs=2, name=name)

            def _invocation():
                # ---- load x (bf16, feature-major, own tokens first) ----
                # own half persistent; other half shares its buffer with x3T
                # (lifetimes: xTh dies after K/V+stats, x3T born in FFN phase)
                xTo = big.tile([P, EC, TOWN], BF16, tag="xo", name="xTo")
                xTh = big.tile([P, EC, TOWN], BF16, tag="shA", name="xTh")
                WDT_ = F8 if FP8 else BF16
                if FP8:
                    x8o = big.tile([P, EC, TOWN], F8, tag="x8o", name="x8o")
                    x8h = big.tile([P, EC, TOWN], F8, tag="x8h", name="x8h")
                for q in range(4):
                    nc.sync.dma_start(xTo[:, q * 2:(q + 1) * 2, :],
                                      xo_view[:, q * 2:(q + 1) * 2, :])
                if FP8:
                    nc.sync.dma_start(x8o, x8o_view)
                wvts = []
                for qd in range(2):
                    wvt = wpool.tile([P, EC, 512], WDT_, tag="wv", bufs=2, name="wvt")
                    wvts.append(wvt)
                nc.sync.dma_start(wvts[0], wv_d.ap()[0])
                for q in range(4):
                    nc.sync.dma_start(xTh[:, q * 2:(q + 1) * 2, :],
                                      xh_view[:, q * 2:(q + 1) * 2, :])
                if FP8:
                    nc.sync.dma_start(x8h, x8h_view)
                nc.sync.dma_start(wvts[1], wv_d.ap()[1])

                def xch(ec, lo, hi):
                    # x chunk [P, lo:hi] in rotated token order (own | oth)
                    if hi <= TOWN:
                        return xTo[:, ec, lo:hi]
                    assert lo >= TOWN
                    return xTh[:, ec, lo - TOWN:hi - TOWN]

                def x8ch(ecs, lo, hi):
                    # fp8 x chunk-pair [P, 2, lo:hi] (rotated token order)
                    if hi <= TOWN:
                        return x8o[:, ecs, lo:hi]
                    assert lo >= TOWN
                    return x8h[:, ecs, lo - TOWN:hi - TOWN]

                def proj_chain(out_ap, wtile, lo, hi, transposed=False):
                    """Accumulate sum_ec w[ec].T @ x[ec] (or x.T @ w for V)
                    into out_ap; fp8 DoubleRow when enabled."""
                    if FP8:
                        for ec in range(0, EC, 2):
                            ecs = slice(ec, ec + 2)
                            if transposed:
                                nc.tensor.matmul(out_ap, x8ch(ecs, lo, hi),
                                                 wtile[:, ecs, :],
                                                 start=(ec == 0), stop=False,
                                                 perf_mode=mybir.MatmulPerfMode.DoubleRow)
                            else:
                                nc.tensor.matmul(out_ap, wtile[:, ecs, :],
                                                 x8ch(ecs, lo, hi),
                                                 start=(ec == 0), stop=False,
                                                 perf_mode=mybir.MatmulPerfMode.DoubleRow)
                    else:
                        for ec in range(EC):
                            if transposed:
                                nc.tensor.matmul(out_ap, xch(ec, lo, hi),
                                                 wtile[:, ec, :],
                                                 start=(ec == 0), stop=False)
                            else:
                                nc.tensor.matmul(out_ap, wtile[:, ec, :],
                                                 xch(ec, lo, hi),
                                                 start=(ec == 0), stop=False)

                # ---- LN1 stats: sum(x), sum(x^2) over features, per token ----
                # stat scratch: 4 rows of [1,512] f32, reused by LN1/LN2/LN3
                # chains (0: nmu, 1: msq/var, 2: rsig, 3: sq/sig)
                scr = rowp.tile([1, 6, 512], F32, tag="srow", bufs=1, name="scr")
                nmu_b = rowp.tile([1, 2, T], BF16, tag="nmub", bufs=1, name="nmu_b")
                # nmu_b[0]: nmu1 bf16 (full T);  nmu_b[1]: sig1 bf16 (full T)
                rsigB = big.tile([P, T], F32, tag="rsigB", name="rsigB")
                rsig_c = rowp.tile([P, SC], F32, tag="rsc", bufs=1, name="rsig_c")
                stat_ps = []
                for th in range(2):
                    ps = pst("ps_s1")
                    pq = pst("ps_q1")
                    for ec in range(EC):
                        xc = xch(ec, th * 512, (th + 1) * 512)
                        xsqc = tmp.tile([P, 512], BF16, tag="xsq", bufs=2, name="xsqc")
                        nc.scalar.activation(xsqc, xc, AF.Square)
                        nc.tensor.matmul(ps, ones_cb, xc,
                                         start=(ec == 0), stop=(ec == EC - 1))
                        nc.tensor.matmul(pq, ones_cb, xsqc,
                                         start=(ec == 0), stop=(ec == EC - 1))
                    stat_ps.append((ps, pq))

                def ln1_chain(th):
                    ts = slice(th * 512, (th + 1) * 512)
                    ps, pq = stat_ps[th]
                    nmu = scr[:, 0, :]
                    nc.vector.tensor_scalar_mul(nmu, ps, -1.0 / E)
                    msq = scr[:, 1, :]
                    nc.vector.tensor_scalar_mul(msq, pq, 1.0 / E)
                    nsq = scr[:, 3, :]
                    nc.vector.scalar_tensor_tensor(nsq, nmu, 1.0, nmu,
                                                   op0=ALU.mult, op1=ALU.mult)
                    var = scr[:, 1, :]
                    nc.vector.tensor_sub(var, msq, nsq)
                    nc.scalar.activation(var, var, AF.Ln, bias=eps1)
                    rsig = scr[:, 2 + 2 * th, :]
                    nc.scalar.activation(rsig, var, AF.Exp, scale=-0.5)
                    nc.vector.tensor_copy(nmu_b[:, 0, ts], nmu)
                    if has_bias:
                        sig = scr[:, 3, :]
                        nc.scalar.activation(sig, var, AF.Exp, scale=0.5)
                        nc.vector.tensor_copy(nmu_b[:, 1, ts], sig)

                def ln1_bcast(th):
                    ts = slice(th * 512, (th + 1) * 512)
                    rsig = scr[:, 2 + 2 * th, :]
                    pb = pmm("pb_rsig1")
                    nc.tensor.matmul(pb, onesP[0:1, :], rsig,
                                     start=True, stop=True)
                    nc.vector.tensor_copy(rsigB[:, ts], pb)
                    pcl = psum.tile([P, 4], F32, tag="pc", bufs=2, name="pcl")
                    for j in range(4):
                        nc.tensor.matmul(pcl[:, j:j + 1],
                                         rsig[:, j * P:(j + 1) * P],
                                         ones1[0:1, :], start=True, stop=True)
                    nc.vector.tensor_copy(rsig_c[:, th * 4:th * 4 + 4], pcl)

                nmu1_r = nmu_b[:, 0, :]
                sig1_r = nmu_b[:, 1, :]

                vsb = big.tile([P, SC, EC, VW], BF16, tag="vsb", name="vsb")

                def v_group(qd, sc):
                    off = 0 if qd == 0 else 97
                    pv = pmm("pv")
                    proj_chain(pv, wvts[qd], sc * P, (sc + 1) * P, transposed=True)
                    nc.tensor.matmul(pv, nmu1_r[:, sc * P:(sc + 1) * P],
                                     csv_r[:, qd * 512:(qd + 1) * 512],
                                     start=False, stop=not has_bias)
                    if has_bias:
                        nc.tensor.matmul(pv, sig1_r[:, sc * P:(sc + 1) * P],
                                         bv_r[:, qd * 512:(qd + 1) * 512],
                                         start=False, stop=True)
                    if FP8:
                        nc.vector.tensor_scalar(
                            vsb[:, sc, :, off:off + 64],
                            pv.rearrange("p (h d) -> p h d", d=64),
                            rsig_c[:, sc:sc + 1], 1.0 / WSCALE,
                            op0=ALU.mult, op1=ALU.mult)
                    else:
                        nc.vector.tensor_scalar_mul(
                            vsb[:, sc, :, off:off + 64],
                            pv.rearrange("p (h d) -> p h d", d=64),
                            rsig_c[:, sc:sc + 1])

                # emission order: chain0 | bcast0 | V-own (PE work covering
                # chain1) | bcast1 | V-oth | Q | K.  The vsb memsets go first
                # (DVE) so they never gate the mm stream.
                nc.vector.memset(vsb[:, :, :, 66:97], 0.0)
                nc.vector.memset(vsb[:, :, :, 64:66], 1.0)
                ln1_chain(0)
                ln1_chain(1)
                ln1_bcast(0)
                for sc in range(4):
                    v_group(0, sc)
                    v_group(1, sc)
                ln1_bcast(1)
                for sc in range(4, SC):
                    v_group(0, sc)
                    v_group(1, sc)

                # ---- Q (own tokens) and K (all tokens), feature-major ----
                qT = big.tile([P, EC, TOWN], BF16, tag="qT", name="qT")
                kT = big.tile([P, EC, T], BF16, tag="kT", name="kT")
                wq_tiles = {}
                wk_tiles = {}
                for hp in range(EC):
                    cs = slice(hp * P, (hp + 1) * P)
                    if hp % 2 == 0:
                        wq2 = wpool.tile([P, 2, EC, P], WDT_, tag="wq", bufs=2,
                                         name="wq2")
                        nc.sync.dma_start(
                            wq2,
                            wq_d.ap()[hp:hp + 2].rearrange("b p c d -> p b c d"))
                        wq_tiles[hp], wq_tiles[hp + 1] = wq2[:, 0], wq2[:, 1]
                        wk2 = wpool.tile([P, 2, EC, P], WDT_, tag="wk", bufs=2,
                                         name="wk2")
                        nc.sync.dma_start(
                            wk2,
                            wk_d.ap()[hp:hp + 2].rearrange("b p c d -> p b c d"))
                        wk_tiles[hp], wk_tiles[hp + 1] = wk2[:, 0], wk2[:, 1]
                    wqt = wq_tiles[hp]
                    pq2 = pmm("pq2")
                    proj_chain(pq2, wqt, 0, TOWN)
                    nc.tensor.matmul(pq2, csq_r[:, cs], nmu1_r[:, 0:TOWN],
                                     start=False, stop=not has_bias)
                    if has_bias:
                        nc.tensor.matmul(pq2, bq_r[:, cs], sig1_r[:, 0:TOWN],
                                         start=False, stop=True)
                    nc.vector.tensor_mul(qT[:, hp, :], pq2, rsigB[:, 0:TOWN])
                    wkt = wk_tiles[hp]
                    for th in range(2):
                        ts = slice(th * 512, (th + 1) * 512)
                        pk = pmm("pk")
                        proj_chain(pk, wkt, th * 512, (th + 1) * 512)
                        nc.tensor.matmul(pk, csk_r[:, cs], nmu1_r[:, ts],
                                         start=False, stop=not has_bias)
                        if has_bias:
                            nc.tensor.matmul(pk, bk_r[:, cs], sig1_r[:, ts],
                                             start=False, stop=True)
                        nc.vector.tensor_mul(kT[:, hp, ts], pk, rsigB[:, ts])

                # FFN weight prefetch: trigger the first two chunk DMAs now so
                # they are resident long before the FFN phase needs them.
                FDT_ = F8 if FP8_FFN else BF16
                fw_tiles = {}

                def fw_fetch(fc):
                    fwt = wpool.tile([P, EC, P], FDT_, tag="fw", bufs=2,
                                     name="fwt")
                    nc.sync.dma_start(fwt, ffw_d.ap()[fc])
                    fw_tiles[fc] = fwt

                fw_fetch(0)
                fw_fetch(1)

                # ---- attention per head pair ----
                a15_src = []
                x2T = big.tile([P, EC, TOWN], BF16, tag="x2T", name="x2T")
                if FP8_FFN:
                    x2f8 = big.tile([P, EC, TOWN], F8, tag="x2f8", name="x2f8")
                ps2 = pst("ps_s2")
                pq2s = pst("ps_q2")
                for hp in range(EC):
                    p_e = tmp.tile([P, SC, 512], BF16, tag="pe", bufs=2, name="p_e")
                    p_o = tmp.tile([P, SC, 512], BF16, tag="po", bufs=2, name="p_o")
                    for sg in range(SC // 2):
                        s0, s1 = 2 * sg, 2 * sg + 1
                        pp = pmm("pp", 1024)
                        nc.tensor.matmul(pp[:, 0:512],
                                         kT[0:64, hp, s0 * P:(s0 + 1) * P],
                                         qT[0:64, hp, :], start=True, stop=True)
                        nc.tensor.matmul(pp[:, 512:1024],
                                         kT[0:64, hp, s1 * P:(s1 + 1) * P],
                                         qT[0:64, hp, :], start=True, stop=True)
                        nc.scalar.activation(
                            p_e[:, s0:s0 + 2, :].rearrange("p a b -> p (a b)"),
                            pp, AF.Exp, scale=SCORE_SCALE / (WSCALE * WSCALE)
                            if FP8 else SCORE_SCALE)
                        pp2 = pmm("pp2", 1024)
                        nc.tensor.matmul(pp2[:, 0:512],
                                         kT[64:128, hp, s0 * P:(s0 + 1) * P],
                                         qT[64:128, hp, :], start=True, stop=True)
                        nc.tensor.matmul(pp2[:, 512:1024],
                                         kT[64:128, hp, s1 * P:(s1 + 1) * P],
                                         qT[64:128, hp, :], start=True, stop=True)
                        nc.scalar.activation(
                            p_o[:, s0:s0 + 2, :].rearrange("p a b -> p (a b)"),
                            pp2, AF.Exp, scale=SCORE_SCALE / (WSCALE * WSCALE)
                            if FP8 else SCORE_SCALE)
                    for half in range(2):
                        h = 2 * hp + half
                        pbase = 64 * half
                        zrow = 64 if half == 0 else 32
                        voff = 0 if half == 0 else 33
                        psl = slice(pbase, pbase + 64)
                        p_sb = p_e if half == 0 else p_o
                        pc = psum.tile([P, 512], F32, tag="pc", bufs=2, name="pc")
                        for sc in range(SC):
                            nc.tensor.matmul(pc, vsb[:, sc, hp, voff:voff + 128],
                                             p_sb[:, sc, :],
                                             start=(sc == 0), stop=(sc == SC - 1))
                        # rz = 1/Z on the Z row, PE-broadcast across partitions
                        zr = tmp.tile([P, 512], F32, tag="zr", bufs=2, name="zr")
                        nc.vector.reciprocal(zr[zrow:zrow + 1, :], pc[zrow:zrow + 1, :])
                        pz = psum.tile([P, 512], F32, tag="pc", bufs=2,
                                       name="pz")
                        nc.tensor.matmul(pz, onesP[zrow:zrow + 1, :],
                                         zr[zrow:zrow + 1, :], start=True, stop=True)
                        # an op may read only ONE non-scalar input from PSUM
                        # (NCC_IBVF027), so rz hops to SBUF first
                        rzB = tmp.tile([P, 512], F32, tag="rzB", bufs=2,
                                       name="rzB")
                        nc.vector.tensor_copy(rzB, pz)
                        ctxn = tmp.tile([P, 512], BF16, tag="ctxn", bufs=2, name="ctxn")
                        nc.vector.tensor_mul(ctxn[psl, :], pc[psl, :], rzB[psl, :])
                        nc.vector.tensor_add(x2T[psl, hp, :], ctxn[psl, :],
                                             xTo[psl, hp, :])
                        if h == H - 1:
                            a15_src.append((p_sb, rzB))
                    if FP8_FFN:
                        nc.vector.tensor_copy(x2f8[:, hp, :], x2T[:, hp, :])
                    x2sqc = tmp.tile([P, 512], BF16, tag="x2sq", bufs=2, name="x2sqc")
                    nc.scalar.activation(x2sqc, x2T[:, hp, :], AF.Square)
                    nc.tensor.matmul(ps2, ones_cb, x2T[:, hp, :],
                                     start=(hp == 0), stop=(hp == EC - 1))
                    nc.tensor.matmul(pq2s, ones_cb, x2sqc,
                                     start=(hp == 0), stop=(hp == EC - 1))

                # ---- LN2 chain (own tokens) ----
                nmu2 = scr[:, 0, :]
                nc.vector.tensor_scalar_mul(nmu2, ps2, -1.0 / E)
                msq2 = scr[:, 1, :]
                nc.vector.tensor_scalar_mul(msq2, pq2s, 1.0 / E)
                var2 = scr[:, 1, :]
                nsq2 = scr[:, 3, :]
                nc.vector.scalar_tensor_tensor(nsq2, nmu2, 1.0, nmu2,
                                               op0=ALU.mult, op1=ALU.mult)
                nc.vector.tensor_sub(var2, msq2, nsq2)
                nc.scalar.activation(var2, var2, AF.Ln, bias=eps1)
                rsig2 = scr[:, 2, :]
                nc.scalar.activation(rsig2, var2, AF.Exp, scale=-0.5)
                nmu2_b = rowp.tile([1, 2, 512], BF16, tag="nmu2b", bufs=1,
                                   name="nmu2_b")
                nc.vector.tensor_copy(nmu2_b[:, 0, :], nmu2)
                if has_bias:
                    sig2 = scr[:, 3, :]
                    nc.scalar.activation(sig2, var2, AF.Exp, scale=0.5)
                    nc.vector.tensor_copy(nmu2_b[:, 1, :], sig2)
                # keep rsig2 in a private row: scr slots 0-3 are reused by the
                # LN3 chain, which still needs rsig2 for its stat transform
                rsig2_keep = rowp.tile([1, 2, 512], F32, tag="rs2k", bufs=1,
                                       name="rsig2_keep")
                nc.vector.tensor_copy(rsig2_keep[:, 0, :], rsig2)
                # rsig2^2 precomputed here; it overlaps the FFN matmuls and
                # shortens the LN3 chain's critical path
                nc.vector.scalar_tensor_tensor(rsig2_keep[:, 1, :], rsig2, 1.0,
                                               rsig2, op0=ALU.mult, op1=ALU.mult)

                # ---- FFN (LN2 folded) + LN3 stats ----
                x3T = big.tile([P, EC, TOWN], BF16, tag="shA", name="x3T")
                ps3 = pst("ps_s3")
                pq3s = pst("ps_q3")
                def ffn_group(fc):
                    if fc not in fw_tiles:
                        fw_fetch(fc)
                    fwt = fw_tiles[fc]
                    py = pmm("py")
                    if FP8_FFN:
                        for ec in range(0, EC, 2):
                            nc.tensor.matmul(
                                py, fwt[:, ec:ec + 2, :], x2f8[:, ec:ec + 2, :],
                                start=(ec == 0), stop=False,
                                perf_mode=mybir.MatmulPerfMode.DoubleRow)
                    else:
                        for ec in range(EC):
                            nc.tensor.matmul(py, fwt[:, ec, :], x2T[:, ec, :],
                                             start=(ec == 0), stop=False)
                    return py

                groups = [ffn_group(0), ffn_group(1)]
                for fc in range(EC):
                    cs = slice(fc * P, (fc + 1) * P)
                    py = groups.pop(0)
                    nc.tensor.matmul(py, csf_r[:, cs], nmu2_b[:, 0, :],
                                     start=False, stop=not has_bias)
                    if has_bias:
                        nc.tensor.matmul(py, bf_r[:, cs], nmu2_b[:, 1, :],
                                         start=False, stop=True)
                    # u = g2 * x2 + py  (x3 = rsig2 (.) u is never
                    # materialized; rsig2 folds into the LN3 rows)
                    nc.vector.scalar_tensor_tensor(x3T[:, fc, :], x2T[:, fc, :],
                                                   g2_p[:, fc:fc + 1], py,
                                                   op0=ALU.mult, op1=ALU.add)
                    x3sq = tmp.tile([P, 512], BF16, tag="x3sq", bufs=2, name="x3sq")
                    nc.scalar.activation(x3sq, x3T[:, fc, :], AF.Square)
                    nc.tensor.matmul(ps3, ones_cb, x3T[:, fc, :],
                                     start=(fc == 0), stop=(fc == EC - 1))
                    nc.tensor.matmul(pq3s, ones_cb, x3sq,
                                     start=(fc == 0), stop=(fc == EC - 1))
                    if fc + 2 < EC:
                        groups.append(ffn_group(fc + 2))

                # ---- head-15 attention map (deferred: fills DVE during FFN) ----
                p15, rz15 = a15_src[0]
                for sg in range(SC // 2):
                    a15s = tmp.tile([P, 2, 512], BF16, tag="a15", bufs=2,
                                    name="a15s")
                    for j in range(2):
                        nc.vector.tensor_mul(a15s[:, j, :],
                                             p15[:, 2 * sg + j, :], rz15)
                    nc.sync.dma_start(a15_view[:, 2 * sg:2 * sg + 2, :], a15s)

                # ---- LN3 chain + apply + out DMA ----
                # stats of u; x3 = rsig2 (.) u, so
                #   nmu3 = -mean(u) * rsig2,  var3 = rsig2^2 * var(u)
                nmu_u = scr[:, 0, :]
                nc.vector.tensor_scalar_mul(nmu_u, ps3, -1.0 / E)
                msq_u = scr[:, 1, :]
                nc.vector.tensor_scalar_mul(msq_u, pq3s, 1.0 / E)
                nsq3 = scr[:, 3, :]
                nc.vector.scalar_tensor_tensor(nsq3, nmu_u, 1.0, nmu_u,
                                               op0=ALU.mult, op1=ALU.mult)
                var_u = scr[:, 1, :]
                nc.vector.tensor_sub(var_u, msq_u, nsq3)
                var3 = scr[:, 1, :]
                nc.vector.tensor_mul(var3, var_u, rsig2_keep[:, 1, :])
                nc.scalar.activation(var3, var3, AF.Ln, bias=eps1)
                rsig3 = scr[:, 2, :]
                nc.scalar.activation(rsig3, var3, AF.Exp, scale=-0.5)
                nmu3 = scr[:, 3, :]
                nc.vector.tensor_mul(nmu3, nmu_u, rsig2_keep[:, 0, :])
                nmrs3 = scr[:, 0, :]
                nc.vector.tensor_mul(nmrs3, nmu3, rsig3)
                nmrs3_b = rowp.tile([1, 512], BF16, tag="nm3b", bufs=1, name="nmrs3_b")
                nc.vector.tensor_copy(nmrs3_b, nmrs3)
                # rc = rsig2 * rsig3 broadcast (replaces separate rsig2B/rsig3B)
                rc3 = scr[:, 2, :]
                nc.vector.tensor_mul(rc3, rsig3, rsig2_keep[:, 0, :])
                pb3 = pmm("pb_rsig3")
                nc.tensor.matmul(pb3, onesP[0:1, :], rc3, start=True, stop=True)
                rsig3B = tmp.tile([P, 512], F32, tag="rs3B", bufs=1, name="rsig3B")
                nc.vector.tensor_copy(rsig3B, pb3)
                for fg in range(EC // 2):
                    ot2 = tmp.tile([P, 2, 512], BF16, tag="lo", bufs=2, name="ot2")
                    for j in range(2):
                        fc = 2 * fg + j
                        cs = slice(fc * P, (fc + 1) * P)
                        # w3B = g3 (x) nmrs3 + b3 (x) ones  (rank-1 psum)
                        pw3 = pmm("pw3")
                        nc.tensor.matmul(pw3, g3_r[:, cs], nmrs3_b,
                                         start=True, stop=not has_bias)
                        if has_bias:
                            nc.tensor.matmul(pw3, b3_r[:, cs], ones_row,
                                             start=False, stop=True)
                        u = tmp.tile([P, 512], F32, tag="lu", bufs=2, name="lu")
                        nc.vector.scalar_tensor_tensor(u, x3T[:, fc, :],
                                                       g3_p[:, fc:fc + 1],
                                                       rsig3B,
                                                       op0=ALU.mult, op1=ALU.mult)
                        nc.vector.tensor_add(ot2[:, j, :], u, pw3)
                    nc.sync.dma_start(out_view[:, 2 * fg:2 * fg + 2, :], ot2)

            if reps > 1:
                with tc.For_i(0, reps, 1):
                    _invocation()
            else:
                _invocation()

    try:
        if not nc.is_finalized():
            nc.finalize()
    finally:
        bacc.get_activation_tables = _orig_tables
    return nc


_NC_CACHE = {}
LAST_RESULT = None


def _prep_host_inputs(x, wq, bq, wk, bk, wv, bv, ffw, ffb,
                      ln1_g, ln1_b, ln2_g, ln2_b, ln3_g, ln3_b):
    f = np.float32
    x = np.asarray(x, f)
    g1 = np.asarray(ln1_g, f)
    b1 = np.asarray(ln1_b, f)
    g2 = np.asarray(ln2_g, f)
    b2 = np.asarray(ln2_b, f)
    ffw = np.asarray(ffw, f)
    ffb = np.asarray(ffb, f)

    def fold(w, bias):
        # w [H,E,DH] -> [E, H*DH] with ln1_g folded; bias_eff = b + b1 @ w
        w = np.asarray(w, f)
        wt = np.transpose(w, (1, 0, 2)).reshape(E, H * DH)
        beff = np.asarray(bias, f).reshape(-1) + b1 @ wt
        wt = wt * g1[:, None]
        return wt, beff

    wqt, bqe = fold(wq, bq)
    wkt, bke = fold(wk, bk)
    wvt, bve = fold(wv, bv)

    # parity-reorder v heads: [0,2,...,14,1,3,...,15]
    perm = list(range(0, H, 2)) + list(range(1, H, 2))
    pidx = np.concatenate([np.arange(h * DH, (h + 1) * DH) for h in perm])
    wvt = wvt[:, pidx]
    bve = bve[pidx]

    # FFN with g2 folded; rank-1 rows
    ffw2 = ffw * g2[:, None]
    csf = g2 + ffw2.sum(axis=0)
    bfe = b2 + b2 @ ffw + ffb

    def pfold(v):  # [E] -> [P, EC] with v[ec*128+p] at [p, ec]
        return np.ascontiguousarray(np.asarray(v, f).reshape(EC, P).T)

    wdt = NF8 if FP8 else NBF
    wsc = WSCALE if FP8 else 1.0

    def blk(wt, d):  # [E, N] -> [N/d, P, EC, d]: w[c*128+p, b*d+j] at [b, p, c, j]
        nb = wt.shape[1] // d
        return np.ascontiguousarray(
            wt.reshape(EC, P, nb, d).transpose(2, 1, 0, 3)).astype(wdt)

    # everything that lands in a projection PSUM lives in the xWSCALE domain
    # (descale folded into exp-scale / V TS / rsig2 broadcast)
    fsc = WSCALE if FP8_FFN else 1.0
    fdt = NF8 if FP8_FFN else NBF

    def fblk(wt, d):
        nb = wt.shape[1] // d
        return np.ascontiguousarray(
            wt.reshape(EC, P, nb, d).transpose(2, 1, 0, 3)).astype(fdt)

    rows = np.concatenate([
        wsc * wqt.sum(axis=0), wsc * bqe, wsc * wkt.sum(axis=0), wsc * bke,
        wsc * wvt.sum(axis=0), wsc * bve, fsc * csf, fsc * bfe,
        np.asarray(ln3_g, f), np.asarray(ln3_b, f),
    ]).reshape(1, 10 * E)

    common = {
        "wq_b": blk(wsc * wqt, P), "wk_b": blk(wsc * wkt, P),
        "wv_b": blk(wsc * wvt, 512),
        "ffw_b": fblk(fsc * ffw2, P),
        "rows_b": np.ascontiguousarray(rows).astype(NBF),
        "cst_p": np.ascontiguousarray(np.concatenate(
            [pfold(fsc * np.asarray(ln2_g, f)), pfold(ln3_g), pfold(ln3_b)],
            axis=1)),
    }
    has_bias = bool(
        np.any(bqe) or np.any(bke) or np.any(bve) or np.any(bfe)
        or np.any(np.asarray(ln3_b, f)))
    return x, common, has_bias


def make_in_maps(x, common):
    in_maps = []
    for core in range(8):
        b, th = core // 2, core % 2
        own = slice(th * TOWN, (th + 1) * TOWN)
        oth = slice((1 - th) * TOWN, (2 - th) * TOWN)
        xTb = np.ascontiguousarray(x[b].T)  # [E, T]
        m = dict(common)
        m["xT_own"] = np.ascontiguousarray(xTb[:, own]).astype(NBF)
        m["xT_oth"] = np.ascontiguousarray(xTb[:, oth]).astype(NBF)
        if FP8:
            m["x8_own"] = np.ascontiguousarray(xTb[:, own]).astype(NF8)
            m["x8_oth"] = np.ascontiguousarray(xTb[:, oth]).astype(NF8)
        in_maps.append(m)
    return in_maps


def assemble(results):
    out = np.empty((B, T, E), np.float32)
    attn = np.empty((B, T, T), np.float32)
    for core in range(8):
        b, th = core // 2, core % 2
        own = slice(th * TOWN, (th + 1) * TOWN)
        oth = slice((1 - th) * TOWN, (2 - th) * TOWN)
        r = results[core]
        out[b, own, :] = np.asarray(r["outT"], np.float32).T
        a = np.asarray(r["attn15T"], np.float32).T  # [t_own, s_local]
        attn[b, own, own] = a[:, 0:TOWN]
        attn[b, own, oth] = a[:, TOWN:T]
    return out, attn


def kernel(x, wq, bq, wk, bk, wv, bv, ffw, ffb,
           ln1_g, ln1_b, ln2_g, ln2_b, ln3_g, ln3_b):
    global LAST_RESULT
    from concourse.bass_utils import run_bass_kernel_spmd

    x, common, has_bias = _prep_host_inputs(
        x, wq, bq, wk, bk, wv, bv, ffw, ffb,
        ln1_g, ln1_b, ln2_g, ln2_b, ln3_g, ln3_b)
    key = (1, has_bias)
    if key not in _NC_CACHE:
        _NC_CACHE[key] = build_nc(1, has_bias)
    nc = _NC_CACHE[key]

    in_maps = make_in_maps(x, common)
    res = run_bass_kernel_spmd(nc, in_maps, core_ids=list(range(8)))
    LAST_RESULT = res
    return assemble(res.results)


def _make_sharded(nc, n_cores=8):
    """jit-compile the SPMD executable for one bass module."""
    import jax
    from jax.sharding import Mesh, PartitionSpec
    from jax.experimental.shard_map import shard_map
    import concourse.mybir as mb
    from concourse import bass2jax

    bass2jax.install_neuronx_cc_hook()
    partition_name = nc.partition_id_tensor.name if nc.partition_id_tensor else None
    in_names, out_names, out_avals, zero_outs = [], [], [], []
    for alloc in nc.m.functions[0].allocations:
        if not isinstance(alloc, mb.MemoryLocationSet):
            continue
        name = alloc.memorylocations[0].name
        if alloc.kind == "ExternalInput":
            if name != partition_name:
                in_names.append(name)
        elif alloc.kind == "ExternalOutput":
            out_names.append(name)
            shape = tuple(alloc.tensor_shape)
            dtype = mb.dt.np(alloc.dtype)
            out_avals.append(jax.core.ShapedArray(shape, dtype))
            zero_outs.append(np.zeros(shape, dtype))
    n_params = len(in_names)
    n_outs = len(out_avals)
    all_names = list(in_names) + list(out_names)
    if partition_name is not None:
        all_names.append(partition_name)

    def _body(*args):
        operands = list(args)
        if partition_name is not None:
            operands.append(bass2jax.partition_id_tensor())
        outs = bass2jax._bass_exec_p.bind(
            *operands,
            out_avals=tuple(out_avals),
            in_names=tuple(all_names),
            out_names=tuple(out_names),
            lowering_input_output_aliases=(),
            sim_require_finite=True,
            sim_require_nnan=True,
            nc=nc,
        )
        return tuple(outs)

    devices = jax.devices()[:n_cores]
    mesh = Mesh(np.asarray(devices), ("core",))
    in_specs = (PartitionSpec("core"),) * (n_params + n_outs)
    out_specs = (PartitionSpec("core"),) * len(out_names)
    sharded = jax.jit(
        shard_map(_body, mesh=mesh, in_specs=in_specs, out_specs=out_specs,
                  check_rep=False),
        keep_unused=True,
    )
    return sharded, in_names, out_names, out_avals, zero_outs


def _time_sharded(sharded, dev_in, dev_zero, iters):
    import time
    import jax

    out_arrs = sharded(*dev_in, *dev_zero)
    jax.block_until_ready(out_arrs)
    times = []
    for _ in range(iters):
        t0 = time.perf_counter()
        out_arrs = sharded(*dev_in, *dev_zero)
        jax.block_until_ready(out_arrs)
        times.append(time.perf_counter() - t0)
    return out_arrs, times


TIMING_REPS = 513


def run_timed(inputs, iters=10):
    """Measure per-invocation HW execution time of the kernel.

    A single dispatch through the axon-tunneled PJRT stack has ~100 ms of
    client<->terminal RPC overhead (measured identical for a trivial
    2-instruction kernel and this full kernel), so single-call wall time
    says nothing about the kernel.  We execute the identical kernel body R
    times back-to-back inside one NEFF (hardware For_i loop; every
    iteration re-reads inputs from DRAM, recomputes, rewrites outputs) and
    report the marginal per-iteration time (t_loop - t_single)/(R - 1),
    which amortizes dispatch overhead and measures steady-state HW
    execution of the full kernel (including the loop's all-engine
    barrier, so it is a conservative estimate).
    """
    import jax

    x, common, has_bias = _prep_host_inputs(**inputs)
    in_maps = make_in_maps(x, common)
    n_cores = 8
    R = TIMING_REPS

    for reps in (1, R):
        if (reps, has_bias) not in _NC_CACHE:
            _NC_CACHE[(reps, has_bias)] = build_nc(reps, has_bias)
    s1, in_names, out_names, out_avals, zero_outs = _make_sharded(
        _NC_CACHE[(1, has_bias)])
    sR, _, _, _, _ = _make_sharded(_NC_CACHE[(R, has_bias)])

    per_core = [[np.asarray(m[name]) for name in in_names] for m in in_maps]
    concat_in = [
        np.concatenate([per_core[c][i] for c in range(n_cores)], axis=0)
        for i in range(len(in_names))
    ]
    concat_zeros = [
        np.zeros((n_cores * z.shape[0], *z.shape[1:]), z.dtype) for z in zero_outs
    ]
    dev_in = [jax.device_put(a) for a in concat_in]
    dev_zero = [jax.device_put(a) for a in concat_zeros]

    out1, t1 = _time_sharded(s1, dev_in, dev_zero, iters)
    outR, tR = _time_sharded(sR, dev_in, dev_zero, iters)

    results1 = [
        {name: np.asarray(out1[i]).reshape(n_cores, *out_avals[i].shape)[c]
         for i, name in enumerate(out_names)}
        for c in range(n_cores)
    ]
    resultsR = [
        {name: np.asarray(outR[i]).reshape(n_cores, *out_avals[i].shape)[c]
         for i, name in enumerate(out_names)}
        for c in range(n_cores)
    ]
    for c in range(n_cores):
        for name in out_names:
            d = np.abs(resultsR[c][name] - results1[c][name]).max()
            assert d < 1e-5, f"loop NEFF mismatch core{c} {name}: {d}"

    t1 = np.asarray(t1)
    tR = np.asarray(tR)
    per_iter = (tR.min() - t1.min()) / (R - 1)
    print(f"single-call wall: min {t1.min()*1e3:.3f} ms (RPC-dominated); "
          f"loop({R}) wall: min {tR.min()*1e3:.3f} ms")
    return assemble(results1), [per_iter]


# revision 9
# speedup vs baseline: 1.1953x; 1.1953x over previous
"""Trainium2 Bass kernel for a 1-layer transformer encoder block (v2).

Reference (B=4, T=1024, E=1024, H=16, DH=64):
    x1 = LN(x);  q/k/v per-head projections of x1
    attn = softmax(q @ k^T * T**-0.5);  ctx = attn @ v (concat heads)
    x2 = LN(x + ctx);  x2 = x2 + x2 @ ffw + ffb;  out = LN(x2)
    also returns attn[:, -1] (head 15's full map)

Sharding: 8 cores = (batch b, token-half).  Each core owns 512 query
tokens of one batch; k/v are computed for the full batch (duplicated
across the pair of cores sharing a batch) so no collectives are needed.
Token order on device is rotated so own tokens are always cols 0:512.

v2 design notes (vs v1):
- ZERO GpSimd ops.  HW gpsimd ops cost multi-us each (two software
  dispatch hops + TIE-FIFO streaming); v1 spent ~200 Pool-engine ops and
  measured 2.4x its simulated time.  All elementwise now on DVE/ACT,
  all partition-broadcasts are K=1 PE matmuls (ones-row x stat-row ->
  PSUM -> one DVE copy to SBUF).
- LN1 and LN2 are FOLDED INTO the QKV / FFN matmuls:
    LN(x) @ W  ==  rsig_t * [ (x @ W')  +  nmu_t (x) colsum(W')
                              + sig_t (x) bias_row ]
  with W' = g (.) W, bias_row = b@W + b_proj, nmu = -mean, sig = 1/rsig,
  all per-token rows computed on device and applied as K=1 rank-1 matmul
  accumulations into the same PSUM group.  The only elementwise cost per
  projection chunk is one DVE op (the *rsig_t scale), and the x1/x2n
  tensors are never materialized.
- x ships in bf16 (matmul operand + residual + stats all read bf16).
- Softmax denominator via ones-columns packed into the V operand
  (unchanged from v1): per head pair the V buffer holds
  [v_even(64) | one_e | one_o | gap(31) | v_odd(64)]; even head ctx
  window cols 0:128 (Z at row 64), odd head cols 33:161 (Z at row 32).
- rsqrt = exp(-0.5*ln(var+eps)); sig = exp(+0.5*ln(var+eps)).
"""

import numpy as np
import ml_dtypes

import concourse.bass as bass
from concourse import bacc
import concourse.mybir as mybir
import concourse.tile as tile

B, T, E, H, DH = 4, 1024, 1024, 16, 64
P = 128
EC = E // P          # 8 feature chunks
SC = T // P          # 8 key-token chunks
TOWN = T // 2        # 512 own query tokens per core
EPS = 1e-5
SCORE_SCALE = T ** -0.5   # 1/32 (fp8: /WSCALE^2 folded in at exp)
VW = 164             # packed v-pair window width (161 used)

F32 = mybir.dt.float32
BF16 = mybir.dt.bfloat16
AF = mybir.ActivationFunctionType
ALU = mybir.AluOpType

NBF = ml_dtypes.bfloat16

FP8 = True                # fp8e4 + DoubleRow for QKV/V projection matmuls
FP8_FFN = False           # FFN stays bf16: fp8 x2 puts ~6% on the direct
                          # residual path into `out` (matmul rel-err == input
                          # vector rel-err; it does not average down)
F8 = mybir.dt.float8e4
NF8 = ml_dtypes.float8_e4m3
WSCALE = 64.0             # fp8 weight scale (weights are ~N(0, 0.02^2))


def _patched_act_tables(module_arch):
    """Restrict Exp/Ln to the one table set containing both, so the
    act-table-load pass emits a single set id instead of thrashing."""
    import concourse.hw_specs as hw_specs
    tabs = hw_specs.get_activation_tables(module_arch)
    both = [k for k, v in tabs.items()
            if AF.Exp in v and AF.Ln in v]
    if not both:
        return tabs
    keep = both[0]
    out = {}
    for k, v in tabs.items():
        out[k] = v if k == keep else (v - {AF.Exp, AF.Ln})
    return out


def build_nc(reps=1, has_bias=True):
    """Build the kernel module.  reps>1 wraps the per-invocation body
    (input DMA + compute + output DMA) in a hardware For_i loop running it
    `reps` times; iterations are identical so outputs are unchanged.  Used
    by run_timed to measure steady-state per-invocation HW time."""
    nc = bacc.Bacc(None, target_bir_lowering=False)
    _orig_tables = bacc.get_activation_tables
    bacc.get_activation_tables = _patched_act_tables

    # ---- dram I/O ----
    x_own_d = nc.dram_tensor("xT_own", [E, TOWN], BF16, kind="ExternalInput")
    x_oth_d = nc.dram_tensor("xT_oth", [E, TOWN], BF16, kind="ExternalInput")
    WDT = F8 if FP8 else BF16
    wq_d = nc.dram_tensor("wq_b", [EC, P, EC, P], WDT, kind="ExternalInput")
    wk_d = nc.dram_tensor("wk_b", [EC, P, EC, P], WDT, kind="ExternalInput")
    wv_d = nc.dram_tensor("wv_b", [2, P, EC, 512], WDT, kind="ExternalInput")  # parity-major
    FDT = F8 if FP8_FFN else BF16
    ffw_d = nc.dram_tensor("ffw_b", [EC, P, EC, P], FDT, kind="ExternalInput")  # g2-folded
    if FP8:
        x8o_d = nc.dram_tensor("x8_own", [E, TOWN], F8, kind="ExternalInput")
        x8h_d = nc.dram_tensor("x8_oth", [E, TOWN], F8, kind="ExternalInput")
        x8o_view = x8o_d.ap().rearrange("(c p) t -> p c t", p=P)
        x8h_view = x8h_d.ap().rearrange("(c p) t -> p c t", p=P)
    # bf16 row constants:
    # [csq, bq_eff, csk, bk_eff, csv, bv_eff, csf, bf_eff, g3, b3]
    rows_d = nc.dram_tensor("rows_b", [1, 10 * E], BF16, kind="ExternalInput")
    # packed per-partition f32 constants: [g2, g3, b3]
    cst_d = nc.dram_tensor("cst_p", [P, 3 * EC], F32, kind="ExternalInput")

    outT_d = nc.dram_tensor("outT", [E, TOWN], BF16, kind="ExternalOutput")
    a15_d = nc.dram_tensor("attn15T", [T, TOWN], BF16, kind="ExternalOutput")

    xo_view = x_own_d.ap().rearrange("(c p) t -> p c t", p=P)
    xh_view = x_oth_d.ap().rearrange("(c p) t -> p c t", p=P)
    out_view = outT_d.ap().rearrange("(c p) t -> p c t", p=P)
    a15_view = a15_d.ap().rearrange("(c p) t -> p c t", p=P)

    with tile.TileContext(nc) as tc:
        with (
            tc.tile_pool(name="const", bufs=1) as const,
            tc.tile_pool(name="big", bufs=1) as big,
            tc.tile_pool(name="wpool", bufs=4) as wpool,
            tc.tile_pool(name="tmp", bufs=2) as tmp,
            tc.tile_pool(name="rowp", bufs=2) as rowp,
            tc.tile_pool(name="psum", bufs=1, space="PSUM") as psum,
        ):
            # ---- constants (outside the timing loop) ----
            ones_cb = const.tile([P, 1], BF16)       # stat matmul lhsT (bf16)
            nc.vector.memset(ones_cb, 1.0)
            onesP = const.tile([P, P], F32)          # bcast lhsT rows (any partition)
            nc.vector.memset(onesP, 1.0)
            ones1 = const.tile([P, 1], F32)          # rhs for row->col transpose
            nc.vector.memset(ones1, 1.0)
            eps1 = const.tile([1, 1], F32)
            nc.vector.memset(eps1, EPS)
            cst = const.tile([P, 3 * EC], F32)
            nc.sync.dma_start(cst, cst_d.ap())
            g2_p = cst[:, 0 * EC:1 * EC]
            g3_p = cst[:, 1 * EC:2 * EC]
            b3_p = cst[:, 2 * EC:3 * EC]
            rows = const.tile([1, 10 * E], BF16)
            nc.sync.dma_start(rows, rows_d.ap())
            csq_r = rows[:, 0 * E:1 * E]
            bq_r = rows[:, 1 * E:2 * E]
            csk_r = rows[:, 2 * E:3 * E]
            bk_r = rows[:, 3 * E:4 * E]
            csv_r = rows[:, 4 * E:5 * E]
            bv_r = rows[:, 5 * E:6 * E]
            csf_r = rows[:, 6 * E:7 * E]
            bf_r = rows[:, 7 * E:8 * E]
            g3_r = rows[:, 8 * E:9 * E]
            b3_r = rows[:, 9 * E:10 * E]
            ones_row = const.tile([1, 512], BF16)
            nc.vector.memset(ones_row, 1.0)
            invsP = const.tile([1, P], F32)   # bcast lhsT carrying 1/WSCALE
            nc.vector.memset(invsP, (1.0 / WSCALE) if FP8_FFN else 1.0)

            def pmm(name, width=512):
                # main accumulation ring (QKV / V / FFN chains + bcasts +
                # attention scores).  Slots are [P,1024] (2 banks); most
                # users take a [P,512] slice.
                t = psum.tile([P, 1024], F32, tag="mm", bufs=2, name=name)
                return t[:, 0:width]

            def pst(name):
                # stat-chain ring [1,512]
                return psum.tile([1, 512], F32, tag="st", bufs=2, name=name)

            def _invocation():
                # ---- load x (bf16, feature-major, own tokens first) ----
                # own half persistent; other half shares its buffer with x3T
                # (lifetimes: xTh dies after K/V+stats, x3T born in FFN phase)
                xTo = big.tile([P, EC, TOWN], BF16, tag="xo", name="xTo")
                xTh = big.tile([P, EC, TOWN], BF16, tag="shA", name="xTh")
                WDT_ = F8 if FP8 else BF16
                if FP8:
                    x8o = big.tile([P, EC, TOWN], F8, tag="x8o", name="x8o")
                    x8h = big.tile([P, EC, TOWN], F8, tag="x8h", name="x8h")
                for q in range(4):
                    nc.sync.dma_start(xTo[:, q * 2:(q + 1) * 2, :],
                                      xo_view[:, q * 2:(q + 1) * 2, :])
                if FP8:
                    nc.sync.dma_start(x8o, x8o_view)
                wvts = []
                for qd in range(2):
                    wvt = wpool.tile([P, EC, 512], WDT_, tag="wv", bufs=2, name="wvt")
                    wvts.append(wvt)
                nc.sync.dma_start(wvts[0], wv_d.ap()[0])
                for q in range(4):
                    nc.sync.dma_start(xTh[:, q * 2:(q + 1) * 2, :],
                                      xh_view[:, q * 2:(q + 1) * 2, :])
                if FP8:
                    nc.sync.dma_start(x8h, x8h_view)
                nc.sync.dma_start(wvts[1], wv_d.ap()[1])

                def xch(ec, lo, hi):
                    # x chunk [P, lo:hi] in rotated token order (own | oth)
                    if hi <= TOWN:
                        return xTo[:, ec, lo:hi]
                    assert lo >= TOWN
                    return xTh[:, ec, lo - TOWN:hi - TOWN]

                def x8ch(ecs, lo, hi):
                    # fp8 x chunk-pair [P, 2, lo:hi] (rotated token order)
                    if hi <= TOWN:
                        return x8o[:, ecs, lo:hi]
                    assert lo >= TOWN
                    return x8h[:, ecs, lo - TOWN:hi - TOWN]

                def proj_chain(out_ap, wtile, lo, hi, transposed=False):
                    """Accumulate sum_ec w[ec].T @ x[ec] (or x.T @ w for V)
                    into out_ap; fp8 DoubleRow when enabled."""
                    if FP8:
                        for ec in range(0, EC, 2):
                            ecs = slice(ec, ec + 2)
                            if transposed:
                                nc.tensor.matmul(out_ap, x8ch(ecs, lo, hi),
                                                 wtile[:, ecs, :],
                                                 start=(ec == 0), stop=False,
                                                 perf_mode=mybir.MatmulPerfMode.DoubleRow)
                            else:
                                nc.tensor.matmul(out_ap, wtile[:, ecs, :],
                                                 x8ch(ecs, lo, hi),
                                                 start=(ec == 0), stop=False,
                                                 perf_mode=mybir.MatmulPerfMode.DoubleRow)
                    else:
                        for ec in range(EC):
                            if transposed:
                                nc.tensor.matmul(out_ap, xch(ec, lo, hi),
                                                 wtile[:, ec, :],
                                                 start=(ec == 0), stop=False)
                            else:
                                nc.tensor.matmul(out_ap, wtile[:, ec, :],
                                                 xch(ec, lo, hi),
                                                 start=(ec == 0), stop=False)

                # ---- LN1 stats: sum(x), sum(x^2) over features, per token ----
                # stat scratch: 4 rows of [1,512] f32, reused by LN1/LN2/LN3
                # chains (0: nmu, 1: msq/var, 2: rsig, 3: sq/sig)
                scr = rowp.tile([1, 6, 512], F32, tag="srow", bufs=1, name="scr")
                nmu_b = rowp.tile([1, 2, T], BF16, tag="nmub", bufs=1, name="nmu_b")
                # nmu_b[0]: nmu1 bf16 (full T);  nmu_b[1]: sig1 bf16 (full T)
                rsigB = big.tile([P, T], F32, tag="rsigB", name="rsigB")
                rsig_c = rowp.tile([P, SC], F32, tag="rsc", bufs=1, name="rsig_c")
                stat_ps = []
                for th in range(2):
                    ps = pst("ps_s1")
                    pq = pst("ps_q1")
                    for ec in range(EC):
                        xc = xch(ec, th * 512, (th + 1) * 512)
                        xsqc = tmp.tile([P, 512], BF16, tag="xsq", bufs=2, name="xsqc")
                        nc.scalar.activation(xsqc, xc, AF.Square)
                        nc.tensor.matmul(ps, ones_cb, xc,
                                         start=(ec == 0), stop=(ec == EC - 1))
                        nc.tensor.matmul(pq, ones_cb, xsqc,
                                         start=(ec == 0), stop=(ec == EC - 1))
                    stat_ps.append((ps, pq))

                def ln1_chain(th):
                    ts = slice(th * 512, (th + 1) * 512)
                    ps, pq = stat_ps[th]
                    nmu = scr[:, 0, :]
                    nc.vector.tensor_scalar_mul(nmu, ps, -1.0 / E)
                    msq = scr[:, 1, :]
                    nc.vector.tensor_scalar_mul(msq, pq, 1.0 / E)
                    nsq = scr[:, 3, :]
                    nc.vector.scalar_tensor_tensor(nsq, nmu, 1.0, nmu,
                                                   op0=ALU.mult, op1=ALU.mult)
                    var = scr[:, 1, :]
                    nc.vector.tensor_sub(var, msq, nsq)
                    nc.scalar.activation(var, var, AF.Ln, bias=eps1)
                    rsig = scr[:, 2 + 2 * th, :]
                    nc.scalar.activation(rsig, var, AF.Exp, scale=-0.5)
                    nc.vector.tensor_copy(nmu_b[:, 0, ts], nmu)
                    if has_bias:
                        sig = scr[:, 3, :]
                        nc.scalar.activation(sig, var, AF.Exp, scale=0.5)
                        nc.vector.tensor_copy(nmu_b[:, 1, ts], sig)

                def ln1_bcast(th):
                    ts = slice(th * 512, (th + 1) * 512)
                    rsig = scr[:, 2 + 2 * th, :]
                    pb = pmm("pb_rsig1")
                    nc.tensor.matmul(pb, onesP[0:1, :], rsig,
                                     start=True, stop=True)
                    nc.vector.tensor_copy(rsigB[:, ts], pb)
                    pcl = psum.tile([P, 4], F32, tag="pc", bufs=2, name="pcl")
                    for j in range(4):
                        nc.tensor.matmul(pcl[:, j:j + 1],
                                         rsig[:, j * P:(j + 1) * P],
                                         ones1[0:1, :], start=True, stop=True)
                    nc.vector.tensor_copy(rsig_c[:, th * 4:th * 4 + 4], pcl)

                nmu1_r = nmu_b[:, 0, :]
                sig1_r = nmu_b[:, 1, :]

                vsb = big.tile([P, SC, EC, VW], BF16, tag="vsb", name="vsb")

                def v_group(qd, sc):
                    off = 0 if qd == 0 else 97
                    pv = pmm("pv")
                    proj_chain(pv, wvts[qd], sc * P, (sc + 1) * P, transposed=True)
                    nc.tensor.matmul(pv, nmu1_r[:, sc * P:(sc + 1) * P],
                                     csv_r[:, qd * 512:(qd + 1) * 512],
                                     start=False, stop=not has_bias)
                    if has_bias:
                        nc.tensor.matmul(pv, sig1_r[:, sc * P:(sc + 1) * P],
                                         bv_r[:, qd * 512:(qd + 1) * 512],
                                         start=False, stop=True)
                    if FP8:
                        nc.vector.tensor_scalar(
                            vsb[:, sc, :, off:off + 64],
                            pv.rearrange("p (h d) -> p h d", d=64),
                            rsig_c[:, sc:sc + 1], 1.0 / WSCALE,
                            op0=ALU.mult, op1=ALU.mult)
                    else:
                        nc.vector.tensor_scalar_mul(
                            vsb[:, sc, :, off:off + 64],
                            pv.rearrange("p (h d) -> p h d", d=64),
                            rsig_c[:, sc:sc + 1])

                # emission order: chain0 | bcast0 | V-own (PE work covering
                # chain1) | bcast1 | V-oth | Q | K.  The vsb memsets go first
                # (DVE) so they never gate the mm stream.
                nc.vector.memset(vsb[:, :, :, 66:97], 0.0)
                nc.vector.memset(vsb[:, :, :, 64:66], 1.0)
                ln1_chain(0)
                ln1_chain(1)
                ln1_bcast(0)
                for sc in range(4):
                    v_group(0, sc)
                    v_group(1, sc)
                ln1_bcast(1)
                for sc in range(4, SC):
                    v_group(0, sc)
                    v_group(1, sc)

                # ---- Q (own tokens) and K (all tokens), feature-major ----
                qT = big.tile([P, EC, TOWN], BF16, tag="qT", name="qT")
                kT = big.tile([P, EC, T], BF16, tag="kT", name="kT")
                for hp in range(EC):
                    cs = slice(hp * P, (hp + 1) * P)
                    wqt = wpool.tile([P, EC, P], WDT_, tag="wq", bufs=2, name="wqt")
                    nc.sync.dma_start(wqt, wq_d.ap()[hp])
                    pq2 = pmm("pq2")
                    proj_chain(pq2, wqt, 0, TOWN)
                    nc.tensor.matmul(pq2, csq_r[:, cs], nmu1_r[:, 0:TOWN],
                                     start=False, stop=not has_bias)
                    if has_bias:
                        nc.tensor.matmul(pq2, bq_r[:, cs], sig1_r[:, 0:TOWN],
                                         start=False, stop=True)
                    nc.vector.tensor_mul(qT[:, hp, :], pq2, rsigB[:, 0:TOWN])
                    wkt = wpool.tile([P, EC, P], WDT_, tag="wk", bufs=2, name="wkt")
                    nc.sync.dma_start(wkt, wk_d.ap()[hp])
                    for th in range(2):
                        ts = slice(th * 512, (th + 1) * 512)
                        pk = pmm("pk")
                        proj_chain(pk, wkt, th * 512, (th + 1) * 512)
                        nc.tensor.matmul(pk, csk_r[:, cs], nmu1_r[:, ts],
                                         start=False, stop=not has_bias)
                        if has_bias:
                            nc.tensor.matmul(pk, bk_r[:, cs], sig1_r[:, ts],
                                             start=False, stop=True)
                        nc.vector.tensor_mul(kT[:, hp, ts], pk, rsigB[:, ts])

                # FFN weight prefetch: trigger the first two chunk DMAs now so
                # they are resident long before the FFN phase needs them.
                FDT_ = F8 if FP8_FFN else BF16
                fw_tiles = {}

                def fw_fetch(fc):
                    fwt = wpool.tile([P, EC, P], FDT_, tag="fw", bufs=2,
                                     name="fwt")
                    nc.sync.dma_start(fwt, ffw_d.ap()[fc])
                    fw_tiles[fc] = fwt

                fw_fetch(0)
                fw_fetch(1)

                # ---- attention per head pair ----
                a15_src = []
                x2T = big.tile([P, EC, TOWN], BF16, tag="x2T", name="x2T")
                if FP8_FFN:
                    x2f8 = big.tile([P, EC, TOWN], F8, tag="x2f8", name="x2f8")
                ps2 = pst("ps_s2")
                pq2s = pst("ps_q2")
                for hp in range(EC):
                    p_e = tmp.tile([P, SC, 512], BF16, tag="pe", bufs=2, name="p_e")
                    p_o = tmp.tile([P, SC, 512], BF16, tag="po", bufs=2, name="p_o")
                    for sg in range(SC // 2):
                        s0, s1 = 2 * sg, 2 * sg + 1
                        pp = pmm("pp", 1024)
                        nc.tensor.matmul(pp[:, 0:512],
                                         kT[0:64, hp, s0 * P:(s0 + 1) * P],
                                         qT[0:64, hp, :], start=True, stop=True)
                        nc.tensor.matmul(pp[:, 512:1024],
                                         kT[0:64, hp, s1 * P:(s1 + 1) * P],
                                         qT[0:64, hp, :], start=True, stop=True)
                        nc.scalar.activation(
                            p_e[:, s0:s0 + 2, :].rearrange("p a b -> p (a b)"),
                            pp, AF.Exp, scale=SCORE_SCALE / (WSCALE * WSCALE)
                            if FP8 else SCORE_SCALE)
                        pp2 = pmm("pp2", 1024)
                        nc.tensor.matmul(pp2[:, 0:512],
                                         kT[64:128, hp, s0 * P:(s0 + 1) * P],
                                         qT[64:128, hp, :], start=True, stop=True)
                        nc.tensor.matmul(pp2[:, 512:1024],
                                         kT[64:128, hp, s1 * P:(s1 + 1) * P],
                                         qT[64:128, hp, :], start=True, stop=True)
                        nc.scalar.activation(
                            p_o[:, s0:s0 + 2, :].rearrange("p a b -> p (a b)"),
                            pp2, AF.Exp, scale=SCORE_SCALE / (WSCALE * WSCALE)
                            if FP8 else SCORE_SCALE)
                    for half in range(2):
                        h = 2 * hp + half
                        pbase = 64 * half
                        zrow = 64 if half == 0 else 32
                        voff = 0 if half == 0 else 33
                        psl = slice(pbase, pbase + 64)
                        p_sb = p_e if half == 0 else p_o
                        pc = psum.tile([P, 512], F32, tag="pc", bufs=2, name="pc")
                        for sc in range(SC):
                            nc.tensor.matmul(pc, vsb[:, sc, hp, voff:voff + 128],
                                             p_sb[:, sc, :],
                                             start=(sc == 0), stop=(sc == SC - 1))
                        # rz = 1/Z on the Z row, PE-broadcast across partitions
                        zr = tmp.tile([P, 512], F32, tag="zr", bufs=2, name="zr")
                        nc.vector.reciprocal(zr[zrow:zrow + 1, :], pc[zrow:zrow + 1, :])
                        pz = psum.tile([P, 512], F32, tag="pc", bufs=2,
                                       name="pz")
                        nc.tensor.matmul(pz, onesP[zrow:zrow + 1, :],
                                         zr[zrow:zrow + 1, :], start=True, stop=True)
                        # an op may read only ONE non-scalar input from PSUM
                        # (NCC_IBVF027), so rz hops to SBUF first
                        rzB = tmp.tile([P, 512], F32, tag="rzB", bufs=2,
                                       name="rzB")
                        nc.vector.tensor_copy(rzB, pz)
                        ctxn = tmp.tile([P, 512], BF16, tag="ctxn", bufs=2, name="ctxn")
                        nc.vector.tensor_mul(ctxn[psl, :], pc[psl, :], rzB[psl, :])
                        nc.vector.tensor_add(x2T[psl, hp, :], ctxn[psl, :],
                                             xTo[psl, hp, :])
                        if h == H - 1:
                            a15_src.append((p_sb, rzB))
                    if FP8_FFN:
                        nc.vector.tensor_copy(x2f8[:, hp, :], x2T[:, hp, :])
                    x2sqc = tmp.tile([P, 512], BF16, tag="x2sq", bufs=2, name="x2sqc")
                    nc.scalar.activation(x2sqc, x2T[:, hp, :], AF.Square)
                    nc.tensor.matmul(ps2, ones_cb, x2T[:, hp, :],
                                     start=(hp == 0), stop=(hp == EC - 1))
                    nc.tensor.matmul(pq2s, ones_cb, x2sqc,
                                     start=(hp == 0), stop=(hp == EC - 1))

                # ---- LN2 chain (own tokens) ----
                nmu2 = scr[:, 0, :]
                nc.vector.tensor_scalar_mul(nmu2, ps2, -1.0 / E)
                msq2 = scr[:, 1, :]
                nc.vector.tensor_scalar_mul(msq2, pq2s, 1.0 / E)
                var2 = scr[:, 1, :]
                nsq2 = scr[:, 3, :]
                nc.vector.scalar_tensor_tensor(nsq2, nmu2, 1.0, nmu2,
                                               op0=ALU.mult, op1=ALU.mult)
                nc.vector.tensor_sub(var2, msq2, nsq2)
                nc.scalar.activation(var2, var2, AF.Ln, bias=eps1)
                rsig2 = scr[:, 2, :]
                nc.scalar.activation(rsig2, var2, AF.Exp, scale=-0.5)
                nmu2_b = rowp.tile([1, 2, 512], BF16, tag="nmu2b", bufs=1,
                                   name="nmu2_b")
                nc.vector.tensor_copy(nmu2_b[:, 0, :], nmu2)
                if has_bias:
                    sig2 = scr[:, 3, :]
                    nc.scalar.activation(sig2, var2, AF.Exp, scale=0.5)
                    nc.vector.tensor_copy(nmu2_b[:, 1, :], sig2)
                # keep rsig2 in a private row: scr slots 0-3 are reused by the
                # LN3 chain, which still needs rsig2 for its stat transform
                rsig2_keep = rowp.tile([1, 2, 512], F32, tag="rs2k", bufs=1,
                                       name="rsig2_keep")
                nc.vector.tensor_copy(rsig2_keep[:, 0, :], rsig2)
                # rsig2^2 precomputed here; it overlaps the FFN matmuls and
                # shortens the LN3 chain's critical path
                nc.vector.scalar_tensor_tensor(rsig2_keep[:, 1, :], rsig2, 1.0,
                                               rsig2, op0=ALU.mult, op1=ALU.mult)

                # ---- FFN (LN2 folded) + LN3 stats ----
                x3T = big.tile([P, EC, TOWN], BF16, tag="shA", name="x3T")
                ps3 = pst("ps_s3")
                pq3s = pst("ps_q3")
                def ffn_group(fc):
                    if fc not in fw_tiles:
                        fw_fetch(fc)
                    fwt = fw_tiles[fc]
                    py = pmm("py")
                    if FP8_FFN:
                        for ec in range(0, EC, 2):
                            nc.tensor.matmul(
                                py, fwt[:, ec:ec + 2, :], x2f8[:, ec:ec + 2, :],
                                start=(ec == 0), stop=False,
                                perf_mode=mybir.MatmulPerfMode.DoubleRow)
                    else:
                        for ec in range(EC):
                            nc.tensor.matmul(py, fwt[:, ec, :], x2T[:, ec, :],
                                             start=(ec == 0), stop=False)
                    return py

                groups = [ffn_group(0), ffn_group(1)]
                for fc in range(EC):
                    cs = slice(fc * P, (fc + 1) * P)
                    py = groups.pop(0)
                    nc.tensor.matmul(py, csf_r[:, cs], nmu2_b[:, 0, :],
                                     start=False, stop=not has_bias)
                    if has_bias:
                        nc.tensor.matmul(py, bf_r[:, cs], nmu2_b[:, 1, :],
                                         start=False, stop=True)
                    # u = g2 * x2 + py  (x3 = rsig2 (.) u is never
                    # materialized; rsig2 folds into the LN3 rows)
                    nc.vector.scalar_tensor_tensor(x3T[:, fc, :], x2T[:, fc, :],
                                                   g2_p[:, fc:fc + 1], py,
                                                   op0=ALU.mult, op1=ALU.add)
                    x3sq = tmp.tile([P, 512], BF16, tag="x3sq", bufs=2, name="x3sq")
                    nc.scalar.activation(x3sq, x3T[:, fc, :], AF.Square)
                    nc.tensor.matmul(ps3, ones_cb, x3T[:, fc, :],
                                     start=(fc == 0), stop=(fc == EC - 1))
                    nc.tensor.matmul(pq3s, ones_cb, x3sq,
                                     start=(fc == 0), stop=(fc == EC - 1))
                    if fc + 2 < EC:
                        groups.append(ffn_group(fc + 2))

                # ---- head-15 attention map (deferred: fills DVE during FFN) ----
                p15, rz15 = a15_src[0]
                for sc in range(SC):
                    a15s = tmp.tile([P, 512], BF16, tag="a15", bufs=1, name="a15s")
                    nc.vector.tensor_mul(a15s, p15[:, sc, :], rz15)
                    nc.sync.dma_start(a15_view[:, sc, :], a15s)

                # ---- LN3 chain + apply + out DMA ----
                # stats of u; x3 = rsig2 (.) u, so
                #   nmu3 = -mean(u) * rsig2,  var3 = rsig2^2 * var(u)
                nmu_u = scr[:, 0, :]
                nc.vector.tensor_scalar_mul(nmu_u, ps3, -1.0 / E)
                msq_u = scr[:, 1, :]
                nc.vector.tensor_scalar_mul(msq_u, pq3s, 1.0 / E)
                nsq3 = scr[:, 3, :]
                nc.vector.scalar_tensor_tensor(nsq3, nmu_u, 1.0, nmu_u,
                                               op0=ALU.mult, op1=ALU.mult)
                var_u = scr[:, 1, :]
                nc.vector.tensor_sub(var_u, msq_u, nsq3)
                var3 = scr[:, 1, :]
                nc.vector.tensor_mul(var3, var_u, rsig2_keep[:, 1, :])
                nc.scalar.activation(var3, var3, AF.Ln, bias=eps1)
                rsig3 = scr[:, 2, :]
                nc.scalar.activation(rsig3, var3, AF.Exp, scale=-0.5)
                nmu3 = scr[:, 3, :]
                nc.vector.tensor_mul(nmu3, nmu_u, rsig2_keep[:, 0, :])
                nmrs3 = scr[:, 0, :]
                nc.vector.tensor_mul(nmrs3, nmu3, rsig3)
                nmrs3_b = rowp.tile([1, 512], BF16, tag="nm3b", bufs=1, name="nmrs3_b")
                nc.vector.tensor_copy(nmrs3_b, nmrs3)
                # rc = rsig2 * rsig3 broadcast (replaces separate rsig2B/rsig3B)
                rc3 = scr[:, 2, :]
                nc.vector.tensor_mul(rc3, rsig3, rsig2_keep[:, 0, :])
                pb3 = pmm("pb_rsig3")
                nc.tensor.matmul(pb3, onesP[0:1, :], rc3, start=True, stop=True)
                rsig3B = tmp.tile([P, 512], F32, tag="rs3B", bufs=1, name="rsig3B")
                nc.vector.tensor_copy(rsig3B, pb3)
                for fc in range(EC):
                    cs = slice(fc * P, (fc + 1) * P)
                    # w3B = g3 (x) nmrs3 + b3 (x) ones   (per-chunk rank-1 psum)
                    pw3 = pmm("pw3")
                    nc.tensor.matmul(pw3, g3_r[:, cs], nmrs3_b,
                                     start=True, stop=not has_bias)
                    if has_bias:
                        nc.tensor.matmul(pw3, b3_r[:, cs], ones_row,
                                         start=False, stop=True)
                    u = tmp.tile([P, 512], F32, tag="lu", bufs=2, name="lu")
                    nc.vector.scalar_tensor_tensor(u, x3T[:, fc, :],
                                                   g3_p[:, fc:fc + 1], rsig3B,
                                                   op0=ALU.mult, op1=ALU.mult)
                    ot = tmp.tile([P, 512], BF16, tag="lo", bufs=2, name="ot")
                    nc.vector.tensor_add(ot, u, pw3)
                    nc.sync.dma_start(out_view[:, fc, :], ot)

            if reps > 1:
                with tc.For_i(0, reps, 1):
                    _invocation()
            else:
                _invocation()

    try:
        if not nc.is_finalized():
            nc.finalize()
    finally:
        bacc.get_activation_tables = _orig_tables
    return nc


_NC_CACHE = {}
LAST_RESULT = None


def _prep_host_inputs(x, wq, bq, wk, bk, wv, bv, ffw, ffb,
                      ln1_g, ln1_b, ln2_g, ln2_b, ln3_g, ln3_b):
    f = np.float32
    x = np.asarray(x, f)
    g1 = np.asarray(ln1_g, f)
    b1 = np.asarray(ln1_b, f)
    g2 = np.asarray(ln2_g, f)
    b2 = np.asarray(ln2_b, f)
    ffw = np.asarray(ffw, f)
    ffb = np.asarray(ffb, f)

    def fold(w, bias):
        # w [H,E,DH] -> [E, H*DH] with ln1_g folded; bias_eff = b + b1 @ w
        w = np.asarray(w, f)
        wt = np.transpose(w, (1, 0, 2)).reshape(E, H * DH)
        beff = np.asarray(bias, f).reshape(-1) + b1 @ wt
        wt = wt * g1[:, None]
        return wt, beff

    wqt, bqe = fold(wq, bq)
    wkt, bke = fold(wk, bk)
    wvt, bve = fold(wv, bv)

    # parity-reorder v heads: [0,2,...,14,1,3,...,15]
    perm = list(range(0, H, 2)) + list(range(1, H, 2))
    pidx = np.concatenate([np.arange(h * DH, (h + 1) * DH) for h in perm])
    wvt = wvt[:, pidx]
    bve = bve[pidx]

    # FFN with g2 folded; rank-1 rows
    ffw2 = ffw * g2[:, None]
    csf = g2 + ffw2.sum(axis=0)
    bfe = b2 + b2 @ ffw + ffb

    def pfold(v):  # [E] -> [P, EC] with v[ec*128+p] at [p, ec]
        return np.ascontiguousarray(np.asarray(v, f).reshape(EC, P).T)

    wdt = NF8 if FP8 else NBF
    wsc = WSCALE if FP8 else 1.0

    def blk(wt, d):  # [E, N] -> [N/d, P, EC, d]: w[c*128+p, b*d+j] at [b, p, c, j]
        nb = wt.shape[1] // d
        return np.ascontiguousarray(
            wt.reshape(EC, P, nb, d).transpose(2, 1, 0, 3)).astype(wdt)

    # everything that lands in a projection PSUM lives in the xWSCALE domain
    # (descale folded into exp-scale / V TS / rsig2 broadcast)
    fsc = WSCALE if FP8_FFN else 1.0
    fdt = NF8 if FP8_FFN else NBF

    def fblk(wt, d):
        nb = wt.shape[1] // d
        return np.ascontiguousarray(
            wt.reshape(EC, P, nb, d).transpose(2, 1, 0, 3)).astype(fdt)

    rows = np.concatenate([
        wsc * wqt.sum(axis=0), wsc * bqe, wsc * wkt.sum(axis=0), wsc * bke,
        wsc * wvt.sum(axis=0), wsc * bve, fsc * csf, fsc * bfe,
        np.asarray(ln3_g, f), np.asarray(ln3_b, f),
    ]).reshape(1, 10 * E)

    common = {
        "wq_b": blk(wsc * wqt, P), "wk_b": blk(wsc * wkt, P),
        "wv_b": blk(wsc * wvt, 512),
        "ffw_b": fblk(fsc * ffw2, P),
        "rows_b": np.ascontiguousarray(rows).astype(NBF),
        "cst_p": np.ascontiguousarray(np.concatenate(
            [pfold(fsc * np.asarray(ln2_g, f)), pfold(ln3_g), pfold(ln3_b)],
            axis=1)),
    }
    has_bias = bool(
        np.any(bqe) or np.any(bke) or np.any(bve) or np.any(bfe)
        or np.any(np.asarray(ln3_b, f)))
    return x, common, has_bias


def make_in_maps(x, common):
    in_maps = []
    for core in range(8):
        b, th = core // 2, core % 2
        own = slice(th * TOWN, (th + 1) * TOWN)
        oth = slice((1 - th) * TOWN, (2 - th) * TOWN)
        xTb = np.ascontiguousarray(x[b].T)  # [E, T]
        m = dict(common)
        m["xT_own"] = np.ascontiguousarray(xTb[:, own]).astype(NBF)
        m["xT_oth"] = np.ascontiguousarray(xTb[:, oth]).astype(NBF)
        if FP8:
            m["x8_own"] = np.ascontiguousarray(xTb[:, own]).astype(NF8)
            m["x8_oth"] = np.ascontiguousarray(xTb[:, oth]).astype(NF8)
        in_maps.append(m)
    return in_maps


def assemble(results):
    out = np.empty((B, T, E), np.float32)
    attn = np.empty((B, T, T), np.float32)
    for core in range(8):
        b, th = core // 2, core % 2
        own = slice(th * TOWN, (th + 1) * TOWN)
        oth = slice((1 - th) * TOWN, (2 - th) * TOWN)
        r = results[core]
        out[b, own, :] = np.asarray(r["outT"], np.float32).T
        a = np.asarray(r["attn15T"], np.float32).T  # [t_own, s_local]
        attn[b, own, own] = a[:, 0:TOWN]
        attn[b, own, oth] = a[:, TOWN:T]
    return out, attn


def kernel(x, wq, bq, wk, bk, wv, bv, ffw, ffb,
           ln1_g, ln1_b, ln2_g, ln2_b, ln3_g, ln3_b):
    global LAST_RESULT
    from concourse.bass_utils import run_bass_kernel_spmd

    x, common, has_bias = _prep_host_inputs(
        x, wq, bq, wk, bk, wv, bv, ffw, ffb,
        ln1_g, ln1_b, ln2_g, ln2_b, ln3_g, ln3_b)
    key = (1, has_bias)
    if key not in _NC_CACHE:
        _NC_CACHE[key] = build_nc(1, has_bias)
    nc = _NC_CACHE[key]

    in_maps = make_in_maps(x, common)
    res = run_bass_kernel_spmd(nc, in_maps, core_ids=list(range(8)))
    LAST_RESULT = res
    return assemble(res.results)


def _make_sharded(nc, n_cores=8):
    """jit-compile the SPMD executable for one bass module."""
    import jax
    from jax.sharding import Mesh, PartitionSpec
    from jax.experimental.shard_map import shard_map
    import concourse.mybir as mb
    from concourse import bass2jax

    bass2jax.install_neuronx_cc_hook()
    partition_name = nc.partition_id_tensor.name if nc.partition_id_tensor else None
    in_names, out_names, out_avals, zero_outs = [], [], [], []
    for alloc in nc.m.functions[0].allocations:
        if not isinstance(alloc, mb.MemoryLocationSet):
            continue
        name = alloc.memorylocations[0].name
        if alloc.kind == "ExternalInput":
            if name != partition_name:
                in_names.append(name)
        elif alloc.kind == "ExternalOutput":
            out_names.append(name)
            shape = tuple(alloc.tensor_shape)
            dtype = mb.dt.np(alloc.dtype)
            out_avals.append(jax.core.ShapedArray(shape, dtype))
            zero_outs.append(np.zeros(shape, dtype))
    n_params = len(in_names)
    n_outs = len(out_avals)
    all_names = list(in_names) + list(out_names)
    if partition_name is not None:
        all_names.append(partition_name)

    def _body(*args):
        operands = list(args)
        if partition_name is not None:
            operands.append(bass2jax.partition_id_tensor())
        outs = bass2jax._bass_exec_p.bind(
            *operands,
            out_avals=tuple(out_avals),
            in_names=tuple(all_names),
            out_names=tuple(out_names),
            lowering_input_output_aliases=(),
            sim_require_finite=True,
            sim_require_nnan=True,
            nc=nc,
        )
        return tuple(outs)

    devices = jax.devices()[:n_cores]
    mesh = Mesh(np.asarray(devices), ("core",))
    in_specs = (PartitionSpec("core"),) * (n_params + n_outs)
    out_specs = (PartitionSpec("core"),) * len(out_names)
    sharded = jax.jit(
        shard_map(_body, mesh=mesh, in_specs=in_specs, out_specs=out_specs,
                  check_rep=False),
        keep_unused=True,
    )
    return sharded, in_names, out_names, out_avals, zero_outs


def _time_sharded(sharded, dev_in, dev_zero, iters):
    import time
    import jax

    out_arrs = sharded(*dev_in, *dev_zero)
    jax.block_until_ready(out_arrs)
    times = []
    for _ in range(iters):
        t0 = time.perf_counter()
        out_arrs = sharded(*dev_in, *dev_zero)
        jax.block_until_ready(out_arrs)
        times.append(time.perf_counter() - t0)
    return out_arrs, times


TIMING_REPS = 513


def run_timed(inputs, iters=10):
    """Measure per-invocation HW execution time of the kernel.

    A single dispatch through the axon-tunneled PJRT stack has ~100 ms of
    client<->terminal RPC overhead (measured identical for a trivial
    2-instruction kernel and this full kernel), so single-call wall time
    says nothing about the kernel.  We execute the identical kernel body R
    times back-to-back inside one NEFF (hardware For_i loop; every
    iteration re-reads inputs from DRAM, recomputes, rewrites outputs) and
    report the marginal per-iteration time (t_loop - t_single)/(R - 1),
    which amortizes dispatch overhead and measures steady-state HW
    execution of the full kernel (including the loop's all-engine
    barrier, so it is a conservative estimate).
    """
    import jax

    x, common, has_bias = _prep_host_inputs(**inputs)
    in_maps = make_in_maps(x, common)
    n_cores = 8
    R = TIMING_REPS

    for reps in (1, R):
        if (reps, has_bias) not in _NC_CACHE:
            _NC_CACHE[(reps, has_bias)] = build_nc(reps, has_bias)
    s1, in_names, out_names, out_avals, zero_outs = _make_sharded(
        _NC_CACHE[(1, has_bias)])
    sR, _, _, _, _ = _make_sharded(_NC_CACHE[(R, has_bias)])

    per_core = [[np.asarray(m[name]) for name in in_names] for m in in_maps]
    concat_in = [
        np.concatenate([per_core[c][i] for c in range(n_cores)], axis=0)
        for i in range(len(in_names))
    ]
    concat_zeros = [
        np.zeros((n_cores * z.shape[0], *z.shape[1:]), z.dtype) for z in zero_outs
    ]
    dev_in = [jax.device_put(a) for a in concat_in]
    dev_zero = [jax.device_put(a) for a in concat_zeros]

    out1, t1 = _time_sharded(s1, dev_in, dev_zero, iters)
    outR, tR = _time_sharded(sR, dev_in, dev_zero, iters)

    results1 = [
        {name: np.asarray(out1[i]).reshape(n_cores, *out_avals[i].shape)[c]
         for i, name in enumerate(out_names)}
        for c in range(n_cores)
    ]
    resultsR = [
        {name: np.asarray(outR[i]).reshape(n_cores, *out_avals[i].shape)[c]
         for i, name in enumerate(out_names)}
        for c in range(n_cores)
    ]
    for c in range(n_cores):
        for name in out_names:
            d = np.abs(resultsR[c][name] - results1[c][name]).max()
            assert d < 1e-5, f"loop NEFF mismatch core{c} {name}: {d}"

    t1 = np.asarray(t1)
    tR = np.asarray(tR)
    per_iter = (tR.min() - t1.min()) / (R - 1)
    print(f"single-call wall: min {t1.min()*1e3:.3f} ms (RPC-dominated); "
          f"loop({R}) wall: min {tR.min()*1e3:.3f} ms")
    return assemble(results1), [per_iter]
